# revision 24
# baseline (speedup 1.0000x reference)
"""ConcatCritic MLP over the B^2 pair grid, Trainium2 Bass/Tile kernel.

out[i, j] = softplus(f(x[i], y[j])) where f is a 4-layer MLP on
concat(x, y). Reference pair grid: pairs[a*n+b] = concat(x[b], y[a]),
scores.reshape(n,n).T -> out.

Key factorization: layer 1 is linear in the concat, so
  h1[a,b] = relu(x[b] @ W1top + y[a] @ W1bot + b1)
with W1top = W1[:128], W1bot = W1[128:]. The [B^2, 256] layer-1 matmul
collapses into two tiny matmuls plus a per-partition broadcast add.

Layout: activations kept transposed [features, batch] so every layer's
matmul (weights stationary as lhsT [K, M]) writes the next layer's rhs
directly: out[m=feat, n=j] = sum_k W[k, m] * hT[k, j].

Score epilogue: L4 matmuls are emitted in groups of 4 steps, each step
targeting PSUM partition 32*(i%4) of a shared bank via tile_position.
The four col positions execute concurrently (col tiling), and grouping
keeps col-positioned matmuls from interleaving with full-array L2/L3
matmuls (which serializes the PE). The drain is one Exp + one Ln over
partitions 0..96 of the bank (engine cost scales with the free dim
only; the junk rows between the four real ones are free) + one
partition-strided DMA to DRAM per 4 steps — no per-step [1,512] ACT
op, no Ln tail. The ACT function table is pinned to
natural_log_exp_and_others via a Bacc subclass; the stock chooser
would thrash exp_and_others <-> natural_log every drain (41 us).

Sharding: core c owns y rows [c*64, (c+1)*64); computes block
V_c[il, j] = f(x[j], y[c*64+il]) of shape [64, 512]. Host gathers
V = concat(V_c) and returns V.T.

Self-contained: hardcodes shapes; imports concourse from the system repo.
"""

import os
import sys

import numpy as np


def _import_concourse():
    try:
        import concourse  # noqa: F401
        return
    except ImportError:
        pass
    for p in ("/opt/trn_rl_repo", "/root/.axon_site/_ro/trn_rl_repo"):
        if os.path.isdir(p) and p not in sys.path:
            sys.path.insert(0, p)
    import concourse  # noqa: F401


_import_concourse()

import concourse.bacc as bacc  # noqa: E402
import concourse.tile as tile  # noqa: E402


class _PinnedTableBacc(bacc.Bacc):
    """Pin the ACT function table to natural_log_exp_and_others (id 6).

    The stock fixpoint pass picks, per activation, the FIRST act_info table
    containing that function: exp -> exp_and_others, ln -> natural_log. A
    loop alternating Exp and Ln therefore reloads tables every iteration
    (measured: 32 loads x 1283 ns = 41 us of ACT time). Table 6 contains
    every function this kernel uses (identity, relu, exp, ln), so one load
    suffices: keep the pass's first insertion, retarget it, drop the rest.
    (Inserted loads carry no semaphores — generate_event_semaphores runs
    earlier — so deletion is sync-safe.)
    """

    _ACT_TABLE_ID = 6  # natural_log_exp_and_others in cayman act_info.json

    def insert_act_table_loads(self):
        super().insert_act_table_loads()
        import concourse.mybir as _mybir
        for blk in self.main_func.blocks:
            first = True
            kept = []
            for inst in blk.instructions:
                if isinstance(inst, _mybir.InstLoadActFuncSet):
                    if not first:
                        continue
                    inst.act_func_set_id = self._ACT_TABLE_ID
                    first = False
                kept.append(inst)
            blk.instructions = kept
from concourse import mybir  # noqa: E402
from concourse.bass_utils import run_bass_kernel_spmd  # noqa: E402

B = 512          # batch (pair-grid side)
D = 128          # input dim per tensor
H = 256          # hidden dim
NCORES = 8
RB = B // NCORES  # 64 y-rows per core
F32 = mybir.dt.float32

# Preamble matmuls (x/y @ W1) run in float32r: fp32 bits in memory,
# single-pass reduced-precision multiply at 1 cycle/row. The hidden layers
# (W2/W3/W4 and h tiles) run in bfloat16: same 1 cycle/row on the PE, but
# weight loads get FWL (2 bf16/cycle), the DVE L1 ops get the 2x packed
# mode, and col-positioned L4 matmuls pass the ISA check (the fp32r
# weight-load path is incompatible with tile_position != 0).
PRE_DT = mybir.dt.float32r
MM_DT = mybir.dt.bfloat16


def _emit(tc, nc, d, out_d):
    AF = mybir.ActivationFunctionType
    OP = mybir.AluOpType
    from contextlib import ExitStack

    with ExitStack() as ctx:
        const = ctx.enter_context(tc.tile_pool(name="const", bufs=1))
        hpool = ctx.enter_context(tc.tile_pool(name="h", bufs=2))
        psum = ctx.enter_context(tc.tile_pool(name="psum", bufs=1, space="PSUM"))

        def load(name, shape, src_ap=None, dt=F32, q=None):
            t = const.tile(list(shape), dt, tag=name, name=name + "_s")
            src = src_ap if src_ap is not None else d[name][:]
            if dt == mybir.dt.float32r:
                # fp32 DRAM bits reinterpreted; bf16 tensors are real bf16 in DRAM.
                src = src.bitcast(dt)
            (q if q is not None else nc.sync).dma_start(out=t[:], in_=src)
            return t

        # The xa chain (xT -> pxa MM -> ACT epilogue -> L1 -> L2MM) is the
        # pipeline-fill critical path: issue its DMAs first on the SP queue.
        # The shorter yb chain loads in parallel on the ACT queue (ACT is
        # idle at kernel start; HWDGE engines are SP and Activation).
        xT = load("xT", (D, B), dt=PRE_DT)
        w1t = load("W1t", (D, H), dt=PRE_DT)
        yT = load("yT", (D, RB), dt=PRE_DT, q=nc.scalar)
        w1b = load("W1b", (D, H), dt=PRE_DT, q=nc.scalar)
        w2 = [load(f"W2_{k}", (128, H), d["W2"][k * 128:(k + 1) * 128, :], MM_DT) for k in range(2)]
        w3 = [load(f"W3_{k}", (128, H), d["W3"][k * 128:(k + 1) * 128, :], MM_DT) for k in range(2)]
        w4 = [load(f"W4_{k}", (128, 1), d["W4"][k * 128:(k + 1) * 128, :], MM_DT) for k in range(2)]
        b1c = [load(f"b1_{k}", (128, 1), d["b1"][k * 128:(k + 1) * 128, :]) for k in range(2)]
        b2c = [load(f"b2_{k}", (128, 1), d["b2"][k * 128:(k + 1) * 128, :]) for k in range(2)]
        b3c = [load(f"b3_{k}", (128, 1), d["b3"][k * 128:(k + 1) * 128, :]) for k in range(2)]
        b4c = load("b4c", (97, 1))

        # ---- preamble: xa[oc] = (x @ W1top)^T + b1 (bias folded here),
        #                yb[oc] = (y_slice @ W1bot)^T
        # Preamble psum tiles borrow the main-loop l2/l3 tags (no extra banks).
        # yb first (tiny copies), xa epilogue on ACT (Identity+bias) so the
        # DVE can start L1(0) as soon as xa lands — shortens pipeline fill.
        # Interleave the two chains fine-grained so each oc=0 producer is as
        # early as possible in its engine stream: L2MM(0) kc0 only needs
        # xa0+yb0, so the fill chain closes ~1 xa-epilogue earlier than the
        # xa-loop-after-yb-loop order.
        xa = []
        yb = []
        for oc in range(2):
            ms = slice(oc * 128, (oc + 1) * 128)
            pxa = psum.tile([128, B], F32, tag=f"l2_{oc}", name=f"pxa{oc}", bufs=2)
            nc.tensor.matmul(pxa[:], lhsT=w1t[:, ms], rhs=xT[:], start=True, stop=True)
            # bf16 so the L1 tensor_scalar qualifies for the DVE 2x packed
            # mode (all non-scalar operands 2-byte; yb rides the exempt
            # scalar port and must stay fp32). xa0 on ACT and xa1 on DVE so
            # the two epilogues run on parallel engines during pipeline
            # fill (serializing both on ACT cost a 3.1us PE gap).
            xat = const.tile([128, B], MM_DT, tag=f"xa{oc}", name=f"xa{oc}")
            if oc == 0:
                nc.scalar.activation(xat[:], pxa[:], AF.Identity, bias=b1c[oc][:, 0:1])
            else:
                nc.vector.tensor_scalar(xat[:], pxa[:], b1c[oc][:, 0:1], 0.0, OP.add, OP.add)
            xa.append(xat)
            pyb = psum.tile([128, RB], F32, tag=f"l3_{oc}", name=f"pyb{oc}", bufs=1)
            nc.tensor.matmul(pyb[:], lhsT=w1b[:, ms], rhs=yT[:], start=True, stop=True)
            ybt = const.tile([128, RB], F32, tag=f"yb{oc}", name=f"yb{oc}")
            nc.vector.tensor_copy(ybt[:], pyb[:])
            yb.append(ybt)

        # Software-pipelined emission. Per step t the PE instruction stream is
        #   L2MM(t) | L4MM(t-2) | L3MM(t-1)
        # so every PE instruction only depends on epilogue work issued in a
        # PREVIOUS step (a full step of slack) — the per-chunk serial chain
        # L2MM->L2epi->L3MM->L3epi->L4MM never stalls the PE.
        # Engine split per step: ACT: 2x L2epi + drain/4; DVE: 2x L1 + 2x L3epi.
        # PSUM banks: l2_0/l2_1 bufs=2 (4) + l3_0/l3_1 bufs=1 (2) + l4 bufs=2
        # (2) = 8 exactly.
        h1s, h2s, h3s = {}, {}, {}
        l4banks = {}

        def emit_l1(i):
            # Both on DVE. (GPSIMD tensor_scalar measures ~7.7us per [128,512]
            # op on HW — 12x DVE — so the Pool engine is useless here.)
            for oc in range(2):
                t = hpool.tile([128, B], MM_DT, tag=f"h1_{oc}", name=f"h1_{oc}_{i}", bufs=3)
                nc.vector.tensor_scalar(
                    t[:], xa[oc][:], yb[oc][:, i:i + 1], 0.0, OP.add, OP.max
                )
                h1s[i, oc] = t

        def emit_l2mm(i):
            for mc in range(2):
                ms = slice(mc * 128, (mc + 1) * 128)
                p = psum.tile([128, B], F32, tag=f"l2_{mc}", name=f"p2_{mc}_{i}", bufs=2)
                for kc in range(2):
                    nc.tensor.matmul(
                        p[:], lhsT=w2[kc][:, ms], rhs=h1s[i, kc][:],
                        start=(kc == 0), stop=(kc == 1),
                    )
                h2s[i, mc] = p  # psum handle; epi converts to SBUF below

        def emit_l2epi(i):
            for mc in range(2):
                p = h2s[i, mc]
                t = hpool.tile([128, B], MM_DT, tag=f"h2_{mc}", name=f"h2_{mc}_{i}")
                nc.scalar.activation(t[:], p[:], AF.Relu, bias=b2c[mc][:, 0:1])
                h2s[i, mc] = t

        def emit_l3mm(i):
            for mc in range(2):
                ms = slice(mc * 128, (mc + 1) * 128)
                p = psum.tile([128, B], F32, tag=f"l3_{mc}", name=f"p3_{mc}_{i}", bufs=1)  # noqa: E501
                for kc in range(2):
                    nc.tensor.matmul(
                        p[:], lhsT=w3[kc][:, ms], rhs=h2s[i, kc][:],
                        start=(kc == 0), stop=(kc == 1),
                    )
                h3s[i, mc] = p

        def emit_l3epi(i):
            for mc in range(2):
                p = h3s[i, mc]
                t = hpool.tile([128, B], MM_DT, tag=f"h3_{mc}", name=f"h3_{mc}_{i}", bufs=6)
                nc.vector.tensor_scalar(t[:], p[:], b3c[mc][:, 0:1], 0.0, OP.add, OP.max)
                h3s[i, mc] = t
            del h2s[i, 0], h2s[i, 1]

        def emit_l4group(g):
            # Steps 4g..4g+3 land on PSUM partitions {0,32,64,96} of one
            # shared bank. Emitting all four col positions back-to-back lets
            # the 32-wide col groups run CONCURRENTLY (col-tiling), so the
            # group costs ~1 N=512 stream per kc chunk instead of 4 — and
            # col-positioned matmuls never interleave with full-array L2/L3
            # matmuls (v2 measured +21us PE serialization from that mix).
            bank = psum.tile([128, B], F32, tag="l4", name=f"p4g_{g}", bufs=2)
            l4banks[g] = bank
            for kc in range(2):
                for j in range(4):
                    i = 4 * g + j
                    pos = 32 * j
                    nc.tensor.matmul(
                        bank[pos:pos + 1, :], lhsT=w4[kc][:], rhs=h3s[i, kc][:],
                        start=(kc == 0), stop=(kc == 1),
                        tile_position=(0, pos),
                    )
            for j in range(4):
                i = 4 * g + j
                del h3s[i, 0], h3s[i, 1]

        def emit_drain(g):
            # Batched score epilogue for steps 4g..4g+3: softplus = ln(1+exp)
            # in two ACT passes over partitions 0..96 of the bank (the four
            # real rows are {0,32,64,96}; the junk rows between cost nothing —
            # ACT cost scales with the free dim only, partitions are parallel;
            # engines forbid partition-strided APs so the stride lives in the
            # final DMA), then one DMA to DRAM.
            bank = l4banks.pop(g)
            se = hpool.tile([97, B], F32, tag="se", name=f"se_{g}")
            nc.scalar.activation(se[:], bank[0:97, :], AF.Exp, bias=b4c[:, 0:1])
            sf = hpool.tile([97, B], F32, tag="sf", name=f"sf_{g}")
            nc.scalar.activation(sf[:], se[:], AF.Ln, bias=1.0)
            nc.sync.dma_start(out=out_d[4 * g:4 * g + 4, :], in_=sf[0:97:32, :])

        # L4 group g (steps 4g..4g+3) is emitted at t = 4g+5: one full
        # iteration after the last h3 of the group lands, so the PE never
        # waits on a same-iteration DVE epilogue.
        emit_l1(0)
        for t in range(RB + 2):
            if t + 1 < RB:
                emit_l1(t + 1)
            if t < RB:
                emit_l2mm(t)
                emit_l2epi(t)
            if t >= 5 and t % 4 == 1:
                emit_l4group((t - 5) // 4)
                emit_drain((t - 5) // 4)
            if t >= 1 and t - 1 < RB:
                emit_l3mm(t - 1)
                emit_l3epi(t - 1)


def _build_program():
    nc = _PinnedTableBacc("TRN2", target_bir_lowering=False, debug=False, enable_asserts=False)
    d = {}
    for name, shape, dt in [
        ("xT", (D, B), F32), ("yT", (D, RB), F32),
        ("W1t", (D, H), F32), ("W1b", (D, H), F32),
        ("W2", (H, H), MM_DT), ("W3", (H, H), MM_DT), ("W4", (H, 1), MM_DT),
        ("b1", (H, 1), F32), ("b2", (H, 1), F32), ("b3", (H, 1), F32),
        ("b4c", (97, 1), F32),
    ]:
        d[name] = nc.dram_tensor(name, list(shape), dt, kind="ExternalInput").ap()
    out_d = nc.dram_tensor("out", [RB, B], F32, kind="ExternalOutput").ap()
    with tile.TileContext(nc) as tc:
        _emit(tc, nc, d, out_d)
    nc.compile()
    return nc


_PROGRAM = None


def _get_program():
    global _PROGRAM
    if _PROGRAM is None:
        _PROGRAM = _build_program()
    return _PROGRAM


def _make_in_maps(x, y, W1, b1, W2, b2, W3, b3, W4, b4):
    import ml_dtypes
    f = np.float32
    bf = ml_dtypes.bfloat16
    xT = np.ascontiguousarray(x.T, dtype=f)
    shared = {
        "xT": xT,
        "W1t": np.ascontiguousarray(W1[:D], dtype=f),
        "W1b": np.ascontiguousarray(W1[D:], dtype=f),
        "W2": np.ascontiguousarray(np.asarray(W2, dtype=f).astype(bf)),
        "W3": np.ascontiguousarray(np.asarray(W3, dtype=f).astype(bf)),
        "W4": np.ascontiguousarray(np.asarray(W4, dtype=f).reshape(H, 1).astype(bf)),
        "b1": np.ascontiguousarray(b1.reshape(H, 1), dtype=f),
        "b2": np.ascontiguousarray(b2.reshape(H, 1), dtype=f),
        "b3": np.ascontiguousarray(b3.reshape(H, 1), dtype=f),
        "b4c": np.full((97, 1), np.asarray(b4, dtype=f).reshape(-1)[0], dtype=f),
    }
    in_maps = []
    for c in range(NCORES):
        m = dict(shared)
        m["yT"] = np.ascontiguousarray(y[c * RB:(c + 1) * RB].T, dtype=f)
        in_maps.append(m)
    return in_maps


def _run(inputs, trace=False, trace_cores=None):
    nc = _get_program()
    in_maps = _make_in_maps(**inputs)
    res = run_bass_kernel_spmd(
        nc, in_maps, list(range(NCORES)), trace=trace, trace_cores=trace_cores,
    )
    V = np.concatenate([res.results[c]["out"] for c in range(NCORES)], axis=0)
    out = np.ascontiguousarray(V.T, dtype=np.float32)
    return out, res


def kernel(**inputs):
    out, _ = _run(inputs, trace=False)
    return out
